# revision 31
# baseline (speedup 1.0000x reference)
"""Trainium2 Bass kernel for nn_AttentionalReadout (segment-softmax pooling).

Algorithm (8-core SPMD, data-parallel over nodes):
  gate_i = tanh(x_i @ W1 + b1) @ W2            (per node; b2 and the segment
                                                max cancel in the softmax)
  e_i    = exp(gate_i)
  out[g] = sum_i e_i x_i / sum_i e_i           (per graph)

Device strategy per core (no on-device transposes, fp8 DMA):
  - nodes are sharded at graph boundaries across the 8 cores; each core
    processes its nodes in blocks of TB 128-node tiles whose graphs fit in a
    G_BLK window (geometry chosen at runtime by _plan).
  - x is shipped TWICE in fp8, pre-tiled on host so every DMA moves ~1 MB of
    per-partition-contiguous data:
      * xT (feature-major, e4m3) feeds layer 1 of the gate MLP as the moving
        operand of a DoubleRowSwInterleave fp8 matmul (W1 stationary,
        host-pre-interleaved A/B pairs with reversed columns so the weight
        load reads contiguously): one 256-deep MM per 512 nodes replaces PE
        transposes + 2 bf16 MMs.
      * xB (node-major, e3m4, 264 B rows for 8 B alignment, with a ones
        column for the denominators) is the moving operand of the pooling
        matmul. e3m4's 4-bit mantissa keeps the weighted-average error
        inside the 2e-2 gate; E stays bf16 (mixed bf16-stationary x
        fp8-moving matmul is exact on PE). The 0/1 one-hot (e3m4) rides in
        the tail of the same tensor.
  - layer 2 of the gate MLP is 4 single-column matmuls per 512-node group
    (u stationary, W2 moving); gate accumulates in PSUM across the block.
  - exp and the E-build (E[p,t,g] = es[p,t] * oh[p,t,g], one broadcast
    tensor_tensor per half-block on DVE) are emitted per half so E of block
    j is ready before pass B of block j runs a block later.
  - emission is software-pipelined one block: pass B of block j-1 (TB
    accumulating 257-column matmuls with E stationary) is emitted before
    pass A of block j. When G_BLK <= 64, even/odd tiles use PE column
    tiling (tile_position (0,0)/(0,64)) and accumulate into partition-
    disjoint halves of one PSUM bank, pairing their issue on the PE.
  - per-block raw [OUTP, 257] partials are DMA'd out; the host sums the
    two column-tiling halves plus partials of graphs straddling block/core
    boundaries, then divides.

Measured on the seed-0 problem: ~273-308 us HW exec (baseline 430 us;
run-to-run HW variance is +-10-15 us), rel err 1.706e-2 (budget 2e-2;
numpy-simulated fp8 error matches exactly). Geometry picked by _plan is
TB=32/G_BLK=40 for this input (worst block span 36 graphs).
"""

import numpy as np
import ml_dtypes

import concourse.bacc as bacc
import concourse.tile as tile
import concourse.mybir as mybir
from concourse.bass_utils import run_bass_kernel_spmd

P = 128            # nodes per tile (partition dim)
XROW = 264         # padded bytes per node row of xB (8-aligned, >= HDIM+1)
HDIM = 256         # node feature dim
NUM_GRAPHS = 8192
N_CORES = 8

_FP = mybir.dt.float32
_BF = mybir.dt.bfloat16
_E4 = mybir.dt.float8e4
_E3 = mybir.dt.float8e3
_NP_BF = np.dtype(ml_dtypes.bfloat16)
_NP_E4 = np.dtype(ml_dtypes.float8_e4m3)
_NP_E3 = np.dtype(ml_dtypes.float8_e3m4)


def _plan(batch):
    """Choose node ranges per core and the uniform block geometry."""
    gpc = NUM_GRAPHS // N_CORES
    bounds = np.searchsorted(
        batch, np.arange(N_CORES + 1, dtype=np.int64) * gpc, side="left"
    ).astype(np.int64)
    t_need = max(1, int(np.ceil(np.diff(bounds).max() / P)))
    for tb, g_blk in [(32, 40), (32, 48), (32, 64), (32, 96), (16, 64), (16, 128), (8, 128)]:
        w = tb * P
        ok = True
        for c in range(N_CORES):
            s, e = int(bounds[c]), int(bounds[c + 1])
            nb = int(np.ceil(max(e - s, 0) / w))
            for j in range(nb):
                lo = s + j * w
                hi = min(lo + w, e)
                if hi <= lo:
                    continue
                if int(batch[hi - 1]) - int(batch[lo]) >= g_blk:
                    ok = False
                    break
            if not ok:
                break
        if ok:
            n_blocks = int(np.ceil(t_need / tb))
            return bounds, tb, g_blk, n_blocks, n_blocks * tb
    raise ValueError("no valid block plan for this batch vector")


def _build_program(T, TB, G_BLK, B):
    """Build the SPMD Bass program (identical across cores)."""
    assert (TB * P) % 512 == 0
    NGRP = TB * P // 512         # 512-node groups per block
    assert NGRP % 2 == 0
    nc = bacc.Bacc("TRN2", target_bir_lowering=False, debug=False)

    xT_d = nc.dram_tensor("xT", [B, P, NGRP * 2 * 512], _E4, kind="ExternalInput")
    xB_d = nc.dram_tensor("xB", [B, P, TB * (XROW + G_BLK)], _E3, kind="ExternalInput")
    w1_d = nc.dram_tensor("w1", [P, 2 * P], _E4, kind="ExternalInput")
    w2_d = nc.dram_tensor("w2", [P, 1], _BF, kind="ExternalInput")
    b1_d = nc.dram_tensor("b1", [P, 1], _FP, kind="ExternalInput")
    OUTP = 64 + G_BLK if G_BLK <= 64 else G_BLK  # col-tiled pass B: 2 halves
    out_d = nc.dram_tensor("out", [B, OUTP, HDIM + 1], _FP, kind="ExternalOutput")

    Tanh = mybir.ActivationFunctionType.Tanh
    Exp = mybir.ActivationFunctionType.Exp
    MUL = mybir.AluOpType.mult
    DR = mybir.MatmulPerfMode.DoubleRowSwInterleave

    with tile.TileContext(nc) as tc:
        with (
            tc.tile_pool(name="const", bufs=1) as const_pool,
            tc.tile_pool(name="xT", bufs=8) as xT_pool,
            tc.tile_pool(name="xB", bufs=8) as xB_pool,
            tc.tile_pool(name="u", bufs=3) as u_pool,
            tc.tile_pool(name="es", bufs=2) as es_pool,
            tc.tile_pool(name="E", bufs=4) as E_pool,
            tc.tile_pool(name="osb", bufs=3) as o_pool,
            tc.tile_pool(name="hp", bufs=2, space="PSUM") as h_pool,
            tc.tile_pool(name="gp", bufs=2, space="PSUM") as gate_pool,
            tc.tile_pool(name="Up", bufs=2, space="PSUM") as U_pool,
        ):
            w1 = const_pool.tile([P, 2, P], _E4)
            nc.sync.dma_start(w1[:], w1_d.ap().rearrange("p (a b) -> p a b", a=2))
            w2 = const_pool.tile([P, 1], _BF)
            nc.sync.dma_start(w2[:], w2_d.ap()[:])
            b1 = const_pool.tile([P, 1], _FP)
            nc.sync.dma_start(b1[:], b1_d.ap()[:])

            NH = NGRP // 2       # tanh halves per block
            TH = TB // NH        # tiles per half
            prev = None          # (E, xB) of block j-1

            def passb_mms(U_ps, E_p, xB_p, t0, t1):
                """Pass-B matmuls for tiles [t0, t1) of the previous block.
                With G_BLK <= 64 even/odd tiles go to different PE column
                groups, accumulating into partition-disjoint halves of one
                PSUM bank (summed on host)."""
                for t in range(t0, t1):
                    if G_BLK <= 64:
                        o0 = 64 * (t % 2)
                        g1 = o0 + G_BLK
                        tp = (0, o0)
                    else:
                        o0, g1, tp = 0, G_BLK, None
                    nc.tensor.matmul(
                        U_ps[o0:g1, :],
                        E_p[:, t, :],
                        xB_p[:, t, : HDIM + 1],
                        start=(t < (2 if G_BLK <= 64 else 1)),
                        stop=(t >= TB - (2 if G_BLK <= 64 else 1)),
                        tile_position=tp,
                    )

            for j in range(B + 1):
                if prev is not None:
                    E_p, xB_p = prev
                    U_ps = U_pool.tile([OUTP, HDIM + 1], _FP)
                    passb_mms(U_ps, E_p, xB_p, 0, TB)

                if j < B:
                    # ---- pass A of block j ----
                    xT = xT_pool.tile([P, NGRP, 2, 512], _E4)
                    nc.sync.dma_start(
                        xT[:],
                        xT_d.ap()[j].rearrange(
                            "p (g a n) -> p g a n", g=NGRP, a=2
                        ),
                    )
                    xB = xB_pool.tile([P, TB * XROW + TB * G_BLK], _E3)
                    nc.sync.dma_start(xB[:], xB_d.ap()[j])
                    xBv = xB[:, : TB * XROW].rearrange("p (t f) -> p t f", t=TB)
                    ohv = xB[:, TB * XROW :].rearrange("p (t g) -> p t g", t=TB)
                    gate_ps = gate_pool.tile([P, TB], _FP)
                    es = es_pool.tile([P, TB], _FP)
                    E = E_pool.tile([P, TB, G_BLK], _BF)
                    for gg in range(NH):
                        h_ps = h_pool.tile([P, 2, 512], _FP)
                        for i2 in range(2):
                            nc.tensor.matmul(
                                h_ps[:, i2, :],
                                w1[:],
                                xT[:, gg * 2 + i2],
                                start=True,
                                stop=True,
                                perf_mode=DR,
                            )
                        u = u_pool.tile([P, 2, 512], _BF)
                        nc.scalar.activation(u[:], h_ps[:], Tanh, bias=b1)
                        for q in range(TH):  # tiles across the 2 groups
                            t = gg * TH + q
                            nc.tensor.matmul(
                                gate_ps[:, t : t + 1],
                                u[:, q // 4, (q % 4) * P : (q % 4 + 1) * P],
                                w2[:],
                                start=True,
                                stop=True,
                            )
                        # exp + E-build per half: E of this block is ready
                        # well before pass B of this block runs next iteration
                        sl = slice(gg * TH, (gg + 1) * TH)
                        nc.scalar.activation(es[:, sl], gate_ps[:, sl], Exp)
                        nc.vector.tensor_tensor(
                            E[:, sl, :],
                            es[:, sl, None].to_broadcast([P, TH, G_BLK]),
                            ohv[:, sl, :],
                            MUL,
                        )
                if prev is not None:
                    out_sb = o_pool.tile([OUTP, HDIM + 1], _FP)
                    nc.vector.tensor_copy(out_sb[:], U_ps[:])
                    nc.sync.dma_start(out_d.ap()[j - 1], out_sb[:])
                prev = (E, xBv) if j < B else None

    nc.compile()
    return nc


def _prep_core(x8T, x8B, batch, bounds, c, T, TB, G_BLK):
    """Per-core padded fp8 shards (both layouts), lidx, per-block bases."""
    s, e = int(bounds[c]), int(bounds[c + 1])
    n = e - s
    B = T // TB
    NGRP = TB * P // 512
    w = TB * P

    # xT: [B, P(k), NGRP, 2(i), 512] with value x[node, k + 128*i]
    xTc = np.zeros((T * P, HDIM), dtype=_NP_E4)
    xTc[:n] = x8T[s:e]
    xTc = np.ascontiguousarray(
        xTc.reshape(B, NGRP, 512, 2, P).transpose(0, 4, 1, 3, 2)
    ).reshape(B, P, NGRP * 2 * 512)

    # xB: [B, P(p), TB, XROW] node-major with ones column, 8B-aligned rows
    xBc = np.zeros((T * P, XROW), dtype=_NP_E3)
    xBc[:n, :HDIM] = x8B[s:e]
    xBc[:n, HDIM] = 1.0
    xBc = np.ascontiguousarray(
        xBc.reshape(B, TB, P, XROW).transpose(0, 2, 1, 3)
    ).reshape(B, P, TB * XROW)

    # one-hot (0/1 in e3m4) + g0
    lidx = np.full(T * P, -1, dtype=np.int64)
    g0 = np.zeros(B, dtype=np.int64)
    bl = batch[s:e]
    for j in range(B):
        lo = j * w
        hi = min(lo + w, n)
        if hi <= lo:
            g0[j] = int(batch[e - 1]) if n > 0 else 0
            continue
        g0[j] = int(bl[lo])
        lidx[lo:hi] = bl[lo:hi] - g0[j]
    ohc = np.zeros((T * P, G_BLK), dtype=_NP_E3)
    valid = lidx >= 0
    ohc[np.nonzero(valid)[0], lidx[valid]] = 1.0
    ohc = np.ascontiguousarray(
        ohc.reshape(B, TB, P, G_BLK).transpose(0, 2, 1, 3)
    ).reshape(B, P, TB * G_BLK)
    xBc = np.ascontiguousarray(np.concatenate([xBc, ohc], axis=2))
    return xTc, xBc, g0


def _make_consts(W1, b1, W2):
    # DoubleRowSwInterleave layout: A/B pairs interleaved per column with
    # columns stored in reverse order (A = W1[:128], B = W1[128:])
    w1c = np.empty((P, 2 * P), dtype=np.float32)
    w1c[:, 0::2] = W1[:P, ::-1]
    w1c[:, 1::2] = W1[P:, ::-1]
    w1c = np.ascontiguousarray(w1c).astype(_NP_E4)
    w2c = W2.reshape(P, 1).astype(_NP_BF)
    b1c = b1.reshape(P, 1).astype(np.float32)
    return w1c, w2c, b1c


_CACHE = {}


def _get_program(T, TB, G_BLK, B):
    key = (T, TB, G_BLK, B)
    if key not in _CACHE:
        _CACHE[key] = _build_program(T, TB, G_BLK, B)
    return _CACHE[key]


def build_in_maps(x, W1, b1, W2, batch):
    """Host-side prep shared by kernel() and the timing harness."""
    batch = np.asarray(batch, dtype=np.int64)
    x = np.asarray(x, dtype=np.float32)
    bounds, TB, G_BLK, B, T = _plan(batch)
    w1c, w2c, b1c = _make_consts(
        np.asarray(W1, dtype=np.float32),
        np.asarray(b1, dtype=np.float32),
        np.asarray(W2, dtype=np.float32),
    )
    x8T = np.clip(x, -240, 240).astype(_NP_E4)
    x8B = x.astype(_NP_E3)
    in_maps, g0s = [], []
    for c in range(N_CORES):
        xTc, xBc, g0 = _prep_core(x8T, x8B, batch, bounds, c, T, TB, G_BLK)
        in_maps.append({"xT": xTc, "xB": xBc, "w1": w1c, "w2": w2c, "b1": b1c})
        g0s.append(g0)
    return in_maps, g0s, (T, TB, G_BLK, B)


def combine(results, g0s, G_BLK):
    """Sum per-block partials into the global output and normalize."""
    U = np.zeros((NUM_GRAPHS + G_BLK, HDIM), dtype=np.float64)
    S = np.zeros(NUM_GRAPHS + G_BLK, dtype=np.float64)
    for out_c, g0 in zip(results, g0s):
        two_half = out_c.shape[1] == 64 + G_BLK
        for j in range(out_c.shape[0]):
            g = int(g0[j])
            dat = out_c[j]
            if two_half:
                dat = dat[:G_BLK] + dat[64 : 64 + G_BLK]
            U[g : g + G_BLK] += dat[:, :HDIM]
            S[g : g + G_BLK] += dat[:, HDIM]
    return (U[:NUM_GRAPHS] / (S[:NUM_GRAPHS, None] + 1e-16)).astype(np.float32)


def kernel(x, W1, b1, W2, b2, batch):
    in_maps, g0s, (T, TB, G_BLK, B) = build_in_maps(x, W1, b1, W2, batch)
    nc = _get_program(T, TB, G_BLK, B)
    res = run_bass_kernel_spmd(nc, in_maps, core_ids=list(range(N_CORES)))
    outs = [res.results[c]["out"] for c in range(N_CORES)]
    return combine(outs, g0s, G_BLK)
